# revision 18
# baseline (speedup 1.0000x reference)
# MoE routing kernel for Trainium2 (Bass/Tile), SPMD over 8 NeuronCores.
#
# Reference computation (B=4, T=2048, D=H=1024, V=8, L=4):
#   h      = gelu(einsum("btd,vdh->btvh", X, W1) + b1)
#   outs   = einsum("btvh,vhk->btvk", h, W2) + b2
#   w      = softmax(op_logits, axis=-1)            # [B, L, V]
#   result = einsum("blv,btvh->bth", w, outs) / L
#
# Strategy:
#   - Host: softmax + mean over L -> wbar[B, V]; fold b2 into a single
#     per-batch combined bias cbias[b] = sum_v wbar[b,v] * b2[v].
#   - Data parallel over tokens: core c owns tokens [c*1024, (c+1)*1024).
#     Each 1024-token shard lies inside a single batch row b, so wbar/cbias
#     are per-core constants (shipped as data => one SPMD program).
#   - Per core, per expert v:
#       MM1:  pre1^T[h, t] = sum_d W1[v][d, h]^T-free X^T[d, t]   (PE)
#       gelu: h_sb[h, t] = Gelu(pre1 + b1[v][h])                  (ACT, PSUM->SBUF)
#       MM2:  out[t, k]  = sum_h h_sb[h, t]-as-lhsT W2[v][h, k]   (PE)
#       acc:  out_acc[t, k] = wbar[v] * out + (cbias | out_acc)   (DVE)
#   - X is pre-transposed on host to [D, BT] so every matmul operand is
#     naturally contraction-major; no on-device transposes anywhere.

import os

import numpy as np
import ml_dtypes

import concourse.bass as bass
import concourse.mybir as mybir
import concourse.tile as tile
from concourse import bacc
from concourse.bass_utils import run_bass_kernel_spmd

N_CORES = 8
P = 128

_DT_MAP = {
    "bf16": mybir.dt.bfloat16,
    "f32r": mybir.dt.float32r,
    "f32": mybir.dt.float32,
}
_NP_DT_MAP = {
    "bf16": ml_dtypes.bfloat16,
    "f32r": np.float32,
    "f32": np.float32,
}


def build_moe_core_program(TC, D, H, V, mode="bf16", act="gelu"):
    """One NeuronCore's program: TC tokens, full V experts."""
    act_func = {
        "gelu": mybir.ActivationFunctionType.Gelu,
        "tanh": mybir.ActivationFunctionType.Tanh,  # sim-only (CoreSim lacks Gelu)
    }[act]
    DT = _DT_MAP[mode]
    f32 = mybir.dt.float32
    DC = D // P          # contraction chunks for MM1
    HC = H // P          # contraction chunks for MM2
    NT = min(512, TC)    # MM1 moving free dim (tokens)
    NK = min(512, H)     # MM2 moving free dim (output cols)
    TT = TC // P         # token tiles of 128

    # Bacc (not plain Bass): its finalize() runs generate_event_semaphores,
    # which splits multi-sem waits — TRN2 allows max 1 wait per instruction.
    nc = bacc.Bacc(trn_type="TRN2")
    x_t = nc.declare_dram_parameter("x_t", [D, TC], DT, isOutput=False)
    w1 = nc.declare_dram_parameter("w1", [V, D, H], DT, isOutput=False)
    w2 = nc.declare_dram_parameter("w2", [V, H, H], DT, isOutput=False)
    b1t = nc.declare_dram_parameter("b1t", [H, V], f32, isOutput=False)
    wbar = nc.declare_dram_parameter("wbar", [P, V], f32, isOutput=False)
    cbias = nc.declare_dram_parameter("cbias", [P, H], f32, isOutput=False)
    out = nc.declare_dram_parameter("out", [TC, H], f32, isOutput=True)

    with tile.TileContext(nc) as tc:
        with (
            tc.tile_pool(name="const", bufs=1) as cpool,
            tc.tile_pool(name="w1p", bufs=2) as w1p,
            tc.tile_pool(name="w2p", bufs=2) as w2p,
            tc.tile_pool(name="hbuf", bufs=1) as hpool,
            tc.tile_pool(name="accp", bufs=1) as accp,
            tc.tile_pool(name="ps1", bufs=4, space="PSUM") as ps1,
            tc.tile_pool(name="ps2", bufs=4, space="PSUM") as ps2,
        ):
            NTH = TC // NT  # token halves
            # Small constants first — negligible bandwidth, needed by the
            # first gelu / accumulate.
            b1_sb = cpool.tile([P, HC, V], f32)
            nc.sync.dma_start(out=b1_sb, in_=b1t.rearrange("(hc p) v -> p hc v", p=P))
            wbar_sb = cpool.tile([P, V], f32)
            nc.sync.dma_start(out=wbar_sb, in_=wbar[:])
            cbias_sb = cpool.tile([P, H], f32)
            nc.sync.dma_start(out=cbias_sb, in_=cbias[:])
            # per-tt output accumulators: each tt's store depends only on its
            # own tile, so final DMAs overlap the last expert's compute.
            out_tiles = [
                accp.tile([P, H], f32, tag=f"acc{tt}", name=f"acc{tt}")
                for tt in range(TT)
            ]
            out_r = out.rearrange("(tt p) k -> p tt k", p=P)

            # All bulk DMAs form one chain with explicit sync deps, ordered by
            # PE consumption: x0, w1[0]_0, x1, w1[0]_1, ..., w2[0], w1[1]...
            # Each link's descriptors only enqueue after the previous link's
            # transfer completes. The HW DGE engines service queues
            # round-robin, so unchained transfers interleave and every chunk
            # finishes at the END of the whole window — chained, the first
            # matmul's operands land in ~1.5us and PE (3.4us of work per
            # chunk-pair) stays ahead of the stream from then on.
            prev_block = None  # last DMA of the previous chain link

            def _dep_on_prev(dma):
                if prev_block is not None:
                    bass._add_dep_helper(
                        dma.ins, prev_block.ins, sync=True,
                        reason="serialize bulk-DMA issue in consumption order",
                    )

            x_tiles = []
            for v in range(V):
                # w1 per-dc tiles; for expert 0, interleaved with the x chunks
                # in consumption order.
                w1_tiles = []
                for dc in range(DC):
                    if v == 0:
                        xt = cpool.tile([P, TC], DT, tag=f"x{dc}", name=f"x{dc}")
                        xdma = nc.sync.dma_start(
                            out=xt, in_=x_t[dc * P:(dc + 1) * P, :]
                        )
                        _dep_on_prev(xdma)
                        prev_block = xdma
                        x_tiles.append(xt)
                    w1t = w1p.tile([P, H], DT, tag=f"w1_{dc}", name=f"w1_{dc}")
                    w1dma = nc.sync.dma_start(
                        out=w1t, in_=w1[v, dc * P:(dc + 1) * P, :]
                    )
                    _dep_on_prev(w1dma)
                    prev_block = w1dma
                    w1_tiles.append(w1t)
                h_sb = hpool.tile([P, HC, TC], DT, tag="h")

                # MM1 + gelu: produce h-major activations h_sb[h, t]
                for hc in range(HC):
                    for th in range(NTH):
                        p1 = ps1.tile([P, NT], f32, tag="p1")
                        for dc in range(DC):
                            nc.tensor.matmul(
                                p1,
                                w1_tiles[dc][:, hc * P:(hc + 1) * P],
                                x_tiles[dc][:, th * NT:(th + 1) * NT],
                                start=(dc == 0),
                                stop=(dc == DC - 1),
                            )
                        nc.scalar.activation(
                            h_sb[:, hc, th * NT:(th + 1) * NT],
                            p1,
                            act_func,
                            bias=b1_sb[:, hc, v:v + 1],
                        )

                # w2 emitted after MM1 + chained, so its transfer can't steal
                # bandwidth from the startup-critical x/w1[0] loads.
                w2_sb = w2p.tile([P, HC, H], DT, tag="w2")
                w2dma = nc.sync.dma_start(
                    out=w2_sb, in_=w2[v].rearrange("(hc p) k -> p hc k", p=P)
                )
                _dep_on_prev(w2dma)
                prev_block = w2dma

                # MM2 + weighted accumulate into out_tiles[tt][t, k]
                for tt in range(TT):
                    for kc in range(H // NK):
                        p2 = ps2.tile([P, NK], f32, tag="p2")
                        for hc in range(HC):
                            nc.tensor.matmul(
                                p2,
                                h_sb[:, hc, tt * P:(tt + 1) * P],
                                w2_sb[:, hc, kc * NK:(kc + 1) * NK],
                                start=(hc == 0),
                                stop=(hc == HC - 1),
                            )
                        ksl = slice(kc * NK, (kc + 1) * NK)
                        in1 = cbias_sb[:, ksl] if v == 0 else out_tiles[tt][:, ksl]
                        nc.vector.scalar_tensor_tensor(
                            out=out_tiles[tt][:, ksl],
                            in0=p2,
                            scalar=wbar_sb[:, v:v + 1],
                            in1=in1,
                            op0=mybir.AluOpType.mult,
                            op1=mybir.AluOpType.add,
                        )
                    if v == V - 1:
                        # store this tt as soon as its accumulation finishes
                        nc.sync.dma_start(out=out_r[:, tt, :], in_=out_tiles[tt][:])
    nc.finalize()  # Bacc: runs wait-splitting + reg alloc passes
    return nc


_prog_cache = {}


def _get_program(mode, TC, D, H, V):
    key = (mode, TC, D, H, V)
    if key not in _prog_cache:
        _prog_cache[key] = build_moe_core_program(TC, D, H, V, mode=mode)
    return _prog_cache[key]


def host_prep(op_logits, token_feats, W1, b1, W2, b2, mode):
    """Shared host-side preprocessing: softmax folding, transpose, cast, shard."""
    op_logits = np.asarray(op_logits, dtype=np.float32)
    token_feats = np.asarray(token_feats, dtype=np.float32)
    W1 = np.asarray(W1, dtype=np.float32)
    b1 = np.asarray(b1, dtype=np.float32)
    W2 = np.asarray(W2, dtype=np.float32)
    b2 = np.asarray(b2, dtype=np.float32)

    B, T, D = token_feats.shape
    V, _, H = W1.shape
    BT = B * T
    TC = BT // N_CORES

    lg = op_logits.astype(np.float64)
    e = np.exp(lg - lg.max(axis=-1, keepdims=True))
    w = e / e.sum(axis=-1, keepdims=True)
    wbar = w.mean(axis=1)                       # [B, V], includes the 1/L
    cbias = wbar @ b2.astype(np.float64)        # [B, H]

    np_dt = _NP_DT_MAP[mode]
    x_t = np.ascontiguousarray(token_feats.reshape(BT, D).T).astype(np_dt)
    w1c = np.ascontiguousarray(W1.astype(np_dt))
    w2c = np.ascontiguousarray(W2.astype(np_dt))
    b1t = np.ascontiguousarray(b1.T.astype(np.float32))

    in_maps = []
    for c in range(N_CORES):
        bc = (c * TC) // T
        in_maps.append({
            "x_t": np.ascontiguousarray(x_t[:, c * TC:(c + 1) * TC]),
            "w1": w1c,
            "w2": w2c,
            "b1t": b1t,
            "wbar": np.ascontiguousarray(
                np.broadcast_to(wbar[bc].astype(np.float32), (P, V))
            ),
            "cbias": np.ascontiguousarray(
                np.broadcast_to(cbias[bc].astype(np.float32), (P, H))
            ),
        })
    return in_maps, (B, T, D, H, V, TC)


LAST_RESULTS = None


def kernel(op_logits, token_feats, W1, b1, W2, b2):
    global LAST_RESULTS
    mode = os.environ.get("MOE_DTYPE", "bf16")
    in_maps, (B, T, D, H, V, TC) = host_prep(
        op_logits, token_feats, W1, b1, W2, b2, mode
    )
    nc = _get_program(mode, TC, D, H, V)
    res = run_bass_kernel_spmd(
        nc,
        in_maps,
        list(range(N_CORES)),
        trace=os.environ.get("MOE_TRACE", "0") == "1",
    )
    LAST_RESULTS = res
    outs = [res.results[c]["out"] for c in range(N_CORES)]
    return np.concatenate(outs, axis=0).reshape(B, T, H).astype(np.float32)


# revision 19
# speedup vs baseline: 1.0621x; 1.0621x over previous
# MoE routing kernel for Trainium2 (Bass/Tile), SPMD over 8 NeuronCores.
#
# Reference computation (B=4, T=2048, D=H=1024, V=8, L=4):
#   h      = gelu(einsum("btd,vdh->btvh", X, W1) + b1)
#   outs   = einsum("btvh,vhk->btvk", h, W2) + b2
#   w      = softmax(op_logits, axis=-1)            # [B, L, V]
#   result = einsum("blv,btvh->bth", w, outs) / L
#
# Strategy:
#   - Host: softmax + mean over L -> wbar[B, V]; fold b2 into a single
#     per-batch combined bias cbias[b] = sum_v wbar[b,v] * b2[v].
#   - Data parallel over tokens: core c owns tokens [c*1024, (c+1)*1024).
#     Each 1024-token shard lies inside a single batch row b, so wbar/cbias
#     are per-core constants (shipped as data => one SPMD program).
#   - Per core, per expert v:
#       MM1:  pre1^T[h, t] = sum_d W1[v][d, h]^T-free X^T[d, t]   (PE)
#       gelu: h_sb[h, t] = Gelu(pre1 + b1[v][h])                  (ACT, PSUM->SBUF)
#       MM2:  out[t, k]  = sum_h h_sb[h, t]-as-lhsT W2[v][h, k]   (PE)
#       acc:  out_acc[t, k] = wbar[v] * out + (cbias | out_acc)   (DVE)
#   - X is pre-transposed on host to [D, BT] so every matmul operand is
#     naturally contraction-major; no on-device transposes anywhere.

import os

import numpy as np
import ml_dtypes

import concourse.bass as bass
import concourse.mybir as mybir
import concourse.tile as tile
from concourse import bacc
from concourse.bass_utils import run_bass_kernel_spmd

N_CORES = 8
P = 128

_DT_MAP = {
    "bf16": mybir.dt.bfloat16,
    "f32r": mybir.dt.float32r,
    "f32": mybir.dt.float32,
}
_NP_DT_MAP = {
    "bf16": ml_dtypes.bfloat16,
    "f32r": np.float32,
    "f32": np.float32,
}


def build_moe_core_program(TC, D, H, V, mode="bf16", act="gelu"):
    """One NeuronCore's program: TC tokens, full V experts."""
    act_func = {
        "gelu": mybir.ActivationFunctionType.Gelu,
        "tanh": mybir.ActivationFunctionType.Tanh,  # sim-only (CoreSim lacks Gelu)
    }[act]
    DT = _DT_MAP[mode]
    f32 = mybir.dt.float32
    DC = D // P          # contraction chunks for MM1
    HC = H // P          # contraction chunks for MM2
    NT = min(512, TC)    # MM1 moving free dim (tokens)
    NK = min(512, H)     # MM2 moving free dim (output cols)
    TT = TC // P         # token tiles of 128

    # Bacc (not plain Bass): its finalize() runs generate_event_semaphores,
    # which splits multi-sem waits — TRN2 allows max 1 wait per instruction.
    nc = bacc.Bacc(trn_type="TRN2")
    x_t = nc.declare_dram_parameter("x_t", [D, TC], DT, isOutput=False)
    w1 = nc.declare_dram_parameter("w1", [V, D, H], DT, isOutput=False)
    w2 = nc.declare_dram_parameter("w2", [V, H, H], DT, isOutput=False)
    b1t = nc.declare_dram_parameter("b1t", [H, V], f32, isOutput=False)
    wbar = nc.declare_dram_parameter("wbar", [P, V], f32, isOutput=False)
    cbias = nc.declare_dram_parameter("cbias", [P, H], f32, isOutput=False)
    out = nc.declare_dram_parameter("out", [TC, H], f32, isOutput=True)

    with tile.TileContext(nc) as tc:
        with (
            tc.tile_pool(name="const", bufs=1) as cpool,
            tc.tile_pool(name="w1p", bufs=2) as w1p,
            tc.tile_pool(name="w2p", bufs=2) as w2p,
            tc.tile_pool(name="hbuf", bufs=1) as hpool,
            tc.tile_pool(name="accp", bufs=1) as accp,
            tc.tile_pool(name="ps1", bufs=4, space="PSUM") as ps1,
            tc.tile_pool(name="ps2", bufs=4, space="PSUM") as ps2,
        ):
            NTH = TC // NT  # token halves
            # Small constants first — negligible bandwidth, needed by the
            # first gelu / accumulate.
            b1_sb = cpool.tile([P, HC, V], f32)
            nc.sync.dma_start(out=b1_sb, in_=b1t.rearrange("(hc p) v -> p hc v", p=P))
            wbar_sb = cpool.tile([P, V], f32)
            nc.sync.dma_start(out=wbar_sb, in_=wbar[:])
            cbias_sb = cpool.tile([P, H], f32)
            nc.sync.dma_start(out=cbias_sb, in_=cbias[:])
            # per-tt output accumulators: each tt's store depends only on its
            # own tile, so final DMAs overlap the last expert's compute.
            out_tiles = [
                accp.tile([P, H], f32, tag=f"acc{tt}", name=f"acc{tt}")
                for tt in range(TT)
            ]
            out_r = out.rearrange("(tt p) k -> p tt k", p=P)

            # x per-dc tiles (2KB DMA lines), unchained — they and w1[0] are
            # the startup-critical set and stream concurrently at full fanout.
            x_tiles = []
            for dc in range(DC):
                xt = cpool.tile([P, TC], DT, tag=f"x{dc}", name=f"x{dc}")
                nc.sync.dma_start(out=xt, in_=x_t[dc * P:(dc + 1) * P, :])
                x_tiles.append(xt)

            # Later weight blocks are chained (2MB granularity) with explicit
            # sync deps: each block's descriptors only enqueue after the
            # previous block's transfer completes. The HW DGE engines service
            # queues round-robin, so unchained later blocks would steal
            # bandwidth from the startup-critical x + w1[0] transfers. (Finer,
            # 256KB-link chaining was tried and starves PE: each sem-gated
            # link adds ~3us of issue latency, throttling the weight stream.)
            prev_block = None  # last DMA of the previous weight block

            def _dep_on_prev(dma):
                if prev_block is not None:
                    bass._add_dep_helper(
                        dma.ins, prev_block.ins, sync=True,
                        reason="serialize weight-block DMA issue",
                    )

            for v in range(V):
                # w1 per-dc tiles: first expert's matmuls begin after the
                # first ~256KB chunks instead of the full 2MB block.
                w1_tiles = []
                for dc in range(DC):
                    w1t = w1p.tile([P, H], DT, tag=f"w1_{dc}", name=f"w1_{dc}")
                    w1dma = nc.sync.dma_start(
                        out=w1t, in_=w1[v, dc * P:(dc + 1) * P, :]
                    )
                    _dep_on_prev(w1dma)
                    w1_tiles.append(w1t)
                prev_block = w1dma
                h_sb = hpool.tile([P, HC, TC], DT, tag="h")

                # MM1 + gelu: produce h-major activations h_sb[h, t]
                for hc in range(HC):
                    for th in range(NTH):
                        p1 = ps1.tile([P, NT], f32, tag="p1")
                        for dc in range(DC):
                            nc.tensor.matmul(
                                p1,
                                w1_tiles[dc][:, hc * P:(hc + 1) * P],
                                x_tiles[dc][:, th * NT:(th + 1) * NT],
                                start=(dc == 0),
                                stop=(dc == DC - 1),
                            )
                        nc.scalar.activation(
                            h_sb[:, hc, th * NT:(th + 1) * NT],
                            p1,
                            act_func,
                            bias=b1_sb[:, hc, v:v + 1],
                        )

                # w2 emitted after MM1 + chained, so its transfer can't steal
                # bandwidth from the startup-critical x/w1[0] loads.
                w2_sb = w2p.tile([P, HC, H], DT, tag="w2")
                w2dma = nc.sync.dma_start(
                    out=w2_sb, in_=w2[v].rearrange("(hc p) k -> p hc k", p=P)
                )
                _dep_on_prev(w2dma)
                prev_block = w2dma

                # MM2 + weighted accumulate into out_tiles[tt][t, k]
                for tt in range(TT):
                    for kc in range(H // NK):
                        p2 = ps2.tile([P, NK], f32, tag="p2")
                        for hc in range(HC):
                            nc.tensor.matmul(
                                p2,
                                h_sb[:, hc, tt * P:(tt + 1) * P],
                                w2_sb[:, hc, kc * NK:(kc + 1) * NK],
                                start=(hc == 0),
                                stop=(hc == HC - 1),
                            )
                        ksl = slice(kc * NK, (kc + 1) * NK)
                        in1 = cbias_sb[:, ksl] if v == 0 else out_tiles[tt][:, ksl]
                        nc.vector.scalar_tensor_tensor(
                            out=out_tiles[tt][:, ksl],
                            in0=p2,
                            scalar=wbar_sb[:, v:v + 1],
                            in1=in1,
                            op0=mybir.AluOpType.mult,
                            op1=mybir.AluOpType.add,
                        )
                    if v == V - 1:
                        # store this tt as soon as its accumulation finishes
                        nc.sync.dma_start(out=out_r[:, tt, :], in_=out_tiles[tt][:])
    nc.finalize()  # Bacc: runs wait-splitting + reg alloc passes
    return nc


_prog_cache = {}


def _get_program(mode, TC, D, H, V):
    key = (mode, TC, D, H, V)
    if key not in _prog_cache:
        _prog_cache[key] = build_moe_core_program(TC, D, H, V, mode=mode)
    return _prog_cache[key]


def host_prep(op_logits, token_feats, W1, b1, W2, b2, mode):
    """Shared host-side preprocessing: softmax folding, transpose, cast, shard."""
    op_logits = np.asarray(op_logits, dtype=np.float32)
    token_feats = np.asarray(token_feats, dtype=np.float32)
    W1 = np.asarray(W1, dtype=np.float32)
    b1 = np.asarray(b1, dtype=np.float32)
    W2 = np.asarray(W2, dtype=np.float32)
    b2 = np.asarray(b2, dtype=np.float32)

    B, T, D = token_feats.shape
    V, _, H = W1.shape
    BT = B * T
    TC = BT // N_CORES

    lg = op_logits.astype(np.float64)
    e = np.exp(lg - lg.max(axis=-1, keepdims=True))
    w = e / e.sum(axis=-1, keepdims=True)
    wbar = w.mean(axis=1)                       # [B, V], includes the 1/L
    cbias = wbar @ b2.astype(np.float64)        # [B, H]

    np_dt = _NP_DT_MAP[mode]
    x_t = np.ascontiguousarray(token_feats.reshape(BT, D).T).astype(np_dt)
    w1c = np.ascontiguousarray(W1.astype(np_dt))
    w2c = np.ascontiguousarray(W2.astype(np_dt))
    b1t = np.ascontiguousarray(b1.T.astype(np.float32))

    in_maps = []
    for c in range(N_CORES):
        bc = (c * TC) // T
        in_maps.append({
            "x_t": np.ascontiguousarray(x_t[:, c * TC:(c + 1) * TC]),
            "w1": w1c,
            "w2": w2c,
            "b1t": b1t,
            "wbar": np.ascontiguousarray(
                np.broadcast_to(wbar[bc].astype(np.float32), (P, V))
            ),
            "cbias": np.ascontiguousarray(
                np.broadcast_to(cbias[bc].astype(np.float32), (P, H))
            ),
        })
    return in_maps, (B, T, D, H, V, TC)


LAST_RESULTS = None


def kernel(op_logits, token_feats, W1, b1, W2, b2):
    global LAST_RESULTS
    mode = os.environ.get("MOE_DTYPE", "bf16")
    in_maps, (B, T, D, H, V, TC) = host_prep(
        op_logits, token_feats, W1, b1, W2, b2, mode
    )
    nc = _get_program(mode, TC, D, H, V)
    res = run_bass_kernel_spmd(
        nc,
        in_maps,
        list(range(N_CORES)),
        trace=os.environ.get("MOE_TRACE", "0") == "1",
    )
    LAST_RESULTS = res
    outs = [res.results[c]["out"] for c in range(N_CORES)]
    return np.concatenate(outs, axis=0).reshape(B, T, H).astype(np.float32)
